# revision 86
# baseline (speedup 1.0000x reference)
"""CapsuleLayer (dynamic routing, 3 iterations) Trainium2 Bass kernel.

Problem: inputs [64, 2048, 16] f32, W [1, 2048, 32, 16, 16] f32
  inputs_hat[b,n,o,d] = sum_i W[n,o,d,i] * inputs[b,n,i]
  3 routing iterations (softmax over o); only the last s/squash matters, and the
  b-update never uses `outputs`, so the whole computation collapses to:
    ihsum[b,n,o] = sum_d ih[b,n,o,d]            (= x . Wsum)
    e1 = exp(ihsum/32); c1 = e1 / sum_o e1
    b2 = ihsum*(1/32 + c1); e2 = exp(b2); r2 = 1/sum_o e2   (c2 = e2*r2)
    s[b,o,d] = sum_n c2[b,n,o] * ih[b,n,o,d]
    out = squash(s)

Sharding: Ni (2048) split 8 ways (256 capsules per core). Routing is local per
(b, n); each core produces a partial s [64, 32, 16] which the host sums and
squashes (tiny: 32K elements).

Per-core device pipeline (all matmuls bf16, PSUM f32), software-pipelined
one block (16 units = 8 quads) ahead:
  pass 1: ihsum via per-capsule matmuls, x stationary (4 capsules col-tiled,
          batch split in halves of 32), batched 16 units per PSUM bank.
  routing (staged across the previous block's pass 2; real-HW constraint:
          gpsimd cannot read PSUM and has no fused STT):
          e1 = ACT exp(ps1/32); z1,r1,r1s=32/z1 on DVE;
          v32 = 32*c1 = e1*r1s (gpsimd TT); t2raw = (v32+1)*ihsum
          (DVE fused STT, PSUM src); e2 = ACT exp(t2raw/32); z2,r2 on DVE.
  pass 2: per unit: ih into PSUM [128, 512]; weighting routes balanced
          under the PE floor: B = DVE STT (psum*r2)*e2, A = ACT copy*r2 +
          DVE TT*e2 (2x mode), C = ACT copy*r2 + gpsimd TT*e2; then a PE
          contraction with a fixed 0/1 selector [128->32] accumulating the
          batch-half h's partial s into ps_s[32h:32h+32] (PSUM, 2 groups).
  epilogue: ACT copies ps_s halves to SBUF, DMA to HBM; host sums the 8
          per-core partials and applies squash.
"""

import os
import sys

import numpy as np
import ml_dtypes

sys.path.insert(0, "/opt/trn_rl_repo")
sys.path.insert(0, "/opt/pypackages")

import concourse.bass as bass
import concourse.mybir as mybir
import concourse.tile as tile
from concourse import bacc
from concourse.bass_utils import run_bass_kernel_spmd

BF16 = mybir.dt.bfloat16
F32 = mybir.dt.float32
AF = mybir.ActivationFunctionType
OP = mybir.AluOpType

B, NI, DI, NO, DO = 64, 2048, 16, 32, 16
NCORES = 8
NL = NI // NCORES            # 256 capsules per core
OD = NO * DO                 # 512
NQ = NL // 4                 # 64 quads (4 capsules each)
NUNITS = NQ * 2              # 128 units: (quad, batch-half)
UNITS_PER_BLOCK = 16         # routing block: 16 units -> psum [128, 512]
NBLOCKS = NUNITS // UNITS_PER_BLOCK   # 8
WCHUNK_Q = 8                 # quads per W dma chunk (32 capsules)
EPS = 1e-7
WARM = int(os.environ.get('K_WARM', '1'))
# per 16 units: route 1 = DVE scalar_tensor_tensor (fused), route 2 =
# ACT copy + GPSIMD multiply, route 3 = ACT copy + DVE multiply (2x mode)
ROUTE1 = frozenset(range(8))
ROUTE2 = frozenset({14})


def _build_program():
    nc = bacc.Bacc("TRN2", target_bir_lowering=False, debug=False)

    x_d = nc.dram_tensor("x", [64, NQ, 2, 128], BF16, kind="ExternalInput").ap()
    w_d = nc.dram_tensor("w", [NQ, 64, OD], BF16, kind="ExternalInput").ap()
    ws_d = nc.dram_tensor("ws", [64, NQ, NO], BF16, kind="ExternalInput").ap()
    es_d = nc.dram_tensor("esel", [128, 32], BF16, kind="ExternalInput").ap()
    s_d = nc.dram_tensor("s_out", [64, OD], F32, kind="ExternalOutput").ap()

    with tile.TileContext(nc) as tc:
        _emit(tc, x_d, w_d, ws_d, es_d, s_d)
    nc.compile()
    return nc


def _emit(tc, x_d, w_d, ws_d, es_d, s_d):
    nc = tc.nc
    from contextlib import ExitStack

    ctx = ExitStack()
    const = ctx.enter_context(tc.tile_pool(name="const", bufs=1))
    wpool = ctx.enter_context(tc.tile_pool(name="w", bufs=4))
    rpool = ctx.enter_context(tc.tile_pool(name="routing", bufs=4))
    spool = ctx.enter_context(tc.tile_pool(name="small", bufs=4))
    e2pool = ctx.enter_context(tc.tile_pool(name="e2", bufs=4))
    r2pool = ctx.enter_context(tc.tile_pool(name="r2", bufs=4))
    tmppool = ctx.enter_context(tc.tile_pool(name="tmp", bufs=12))
    ps1pool = ctx.enter_context(tc.tile_pool(name="ps1", bufs=1, space="PSUM"))
    psihpool = ctx.enter_context(tc.tile_pool(name="psih", bufs=6, space="PSUM"))
    psspool = ctx.enter_context(tc.tile_pool(name="pss", bufs=1, space="PSUM"))

    # resident inputs
    # x: block-diagonal stationary per (quad, half): [64 = (4n,16i), 128 = (4n,32b)]
    # loaded in per-block slices, interleaved with W so block k's inputs all
    # arrive ~2 block-periods before use
    x_sb = const.tile([64, NQ, 2, 128], BF16)
    ws_sb = const.tile([64, NQ, NO], BF16)
    es_sb = const.tile([128, 32], BF16)
    warm_sb = const.tile([64, 512], BF16)
    w_tiles = [None] * NBLOCKS

    def emit_dma_xws(blk, split=False):
        q0 = blk * WCHUNK_Q
        if split:
            # halve the first transfer so block 0's first pass-1 matmuls
            # (quads q0..q0+3) start as early as possible
            hq = WCHUNK_Q // 2
            nc.sync.dma_start(x_sb[:, q0:q0 + hq], x_d[:, q0:q0 + hq])
            nc.sync.dma_start(ws_sb[:, q0:q0 + hq], ws_d[:, q0:q0 + hq])
            nc.sync.dma_start(x_sb[:, q0 + hq:q0 + WCHUNK_Q],
                              x_d[:, q0 + hq:q0 + WCHUNK_Q])
            nc.sync.dma_start(ws_sb[:, q0 + hq:q0 + WCHUNK_Q],
                              ws_d[:, q0 + hq:q0 + WCHUNK_Q])
            return
        nc.sync.dma_start(ws_sb[:, q0:q0 + WCHUNK_Q], ws_d[:, q0:q0 + WCHUNK_Q])
        nc.sync.dma_start(x_sb[:, q0:q0 + WCHUNK_Q], x_d[:, q0:q0 + WCHUNK_Q])

    def emit_dma_w(blk):
        q0 = blk * WCHUNK_Q
        w_tile = wpool.tile([64, WCHUNK_Q, OD], BF16, tag="wt")
        nc.sync.dma_start(
            w_tile[:], w_d[q0:q0 + WCHUNK_Q].rearrange("q p f -> p q f")
        )
        w_tiles[blk] = w_tile

    def emit_dma(blk):
        emit_dma_xws(blk)
        emit_dma_w(blk)

    # s accumulator psum, lives across the whole pass 2.
    # [64 b, 512 = (d,o)]; batch-half h units accumulate partitions 32h:32h+32
    # (disjoint partition ranges -> subtile deps let the epilogue drain halves)
    ps_s = psspool.tile([64, OD], F32)
    s_written = [False, False]
    e_emitted = [0, 0]

    # PE warm-up/filler matmuls: keep the tensor engine busy (and its p-state
    # ramp hot) during pipeline fill. Only legal before the first real E.
    def warm_mm(cols):
        nc.tensor.matmul(
            ps_s[:, 0:cols],
            lhsT=warm_sb[:, 0:64],
            rhs=warm_sb[:, 0:cols],
            start=True, stop=True,
        )

    e2_blocks = [None] * NBLOCKS
    r2_blocks = [None] * NBLOCKS

    # deferred E-contractions: list of (u, tmp_tile)
    E_LAG = 11
    pending_e = []

    def flush_e(u_final, lag=None):
        lag = E_LAG if lag is None else lag
        while pending_e and (len(pending_e) > lag or u_final):
            eu, etmp = pending_e.pop(0)
            h = eu % 2
            e_emitted[h] += 1
            nc.tensor.matmul(
                ps_s[32 * h:32 * (h + 1), :],
                lhsT=es_sb[:],
                rhs=etmp.rearrange("p d o -> p (d o)"),
                start=not s_written[h], stop=(e_emitted[h] == NUNITS // 2),
            )
            s_written[h] = True

    # -------- software pipeline over blocks --------
    # Routing is staged in (blk, chunk) pieces so each in-order engine queue
    # matches data readiness. Block 0 uses 4-unit chunks to cut pipeline-fill
    # latency; steady blocks use one 16-unit chunk per stage:
    #   pass1(b):      ihsum matmuls (PE)
    #   e1_(b,c):      exp (ACT)
    #   z1r1(b,c):     reduce+recip (DVE)
    #   mid(b,c):      u1, t2 (Pool STT)
    #   e2_(b,c):      exp (ACT)
    #   z2r2(b,c):     reduce+recip (DVE)
    ps1_blocks = [None] * NBLOCKS
    e1_t, r1_t, e2_t, r2_t = {}, {}, {}, {}

    def chunks_of(blk):
        if blk == 0:
            return [(0, 4), (4, 4), (8, 8)]
        if blk == 1:
            return [(0, 8), (8, 8)]
        return [(0, 16)]

    def ckey(blk, j):
        if blk == 0:
            c0 = (j // 4) * 4 if j < 8 else 8
        elif blk == 1:
            c0 = (j // 8) * 8
        else:
            c0 = 0
        return (blk, c0), j - c0

    def pass1(blk):
        ps1 = ps1pool.tile([128, UNITS_PER_BLOCK * NO], F32)
        for j in range(UNITS_PER_BLOCK):
            u = blk * UNITS_PER_BLOCK + j
            q, h = u // 2, u % 2
            nc.tensor.matmul(
                ps1[:, 32 * j:32 * (j + 1)],
                lhsT=x_sb[:, q, h, :],
                rhs=ws_sb[:, q, :],
                start=True, stop=True,
            )
        ps1_blocks[blk] = ps1

    def ps1v(blk, c0, cs):
        return ps1_blocks[blk].rearrange(
            "p (j o) -> p j o", o=NO)[:, c0:c0 + cs, :]

    def e1_(blk, c0, cs):
        e1 = rpool.tile([128, cs, NO], BF16, tag=f"e1s{cs}c{c0}")
        nc.scalar.activation(e1[:], ps1v(blk, c0, cs), AF.Exp, scale=1.0 / 32.0)
        e1_t[(blk, c0)] = e1

    def z1r1(blk, c0, cs):
        e1 = e1_t[(blk, c0)]
        z1 = spool.tile([128, cs], F32, tag=f"z1s{cs}c{c0}")
        nc.vector.tensor_reduce(z1[:], e1[:], axis=mybir.AxisListType.X, op=OP.add)
        r1 = spool.tile([128, cs], F32, tag=f"r1s{cs}c{c0}")
        nc.vector.reciprocal(r1[:], z1[:])
        r1s = spool.tile([128, cs], F32, tag=f"r1x{cs}c{c0}")
        nc.vector.tensor_scalar_mul(r1s[:], r1[:], 32.0)
        r1_t[(blk, c0)] = r1s

    def mid(blk, c0, cs):
        # v32 = 32*c1 = e1 * (32*r1)  (gpsimd TT: the only legal Pool form)
        # t2raw = (v32 + 1) * ihsum   (DVE fused STT, reads ihsum from PSUM)
        # e2 = exp(t2raw / 32) = exp(ihsum*(1/32 + c1))
        e1 = e1_t[(blk, c0)]
        r1_b = r1_t[(blk, c0)][:, :, None].to_broadcast((128, cs, NO))
        v32 = rpool.tile([128, cs, NO], BF16, tag=f"u1s{cs}c{c0}")
        nc.gpsimd.tensor_tensor(v32[:], e1[:], r1_b, op=OP.mult)
        t2 = rpool.tile([128, cs, NO], BF16, tag=f"t2s{cs}c{c0}")
        nc.vector.scalar_tensor_tensor(t2[:], v32[:], 1.0, ps1v(blk, c0, cs),
                                       op0=OP.add, op1=OP.mult)
        e1_t[(blk, c0, "t2")] = t2

    def e2_(blk, c0, cs):
        t2 = e1_t[(blk, c0, "t2")]
        e2 = e2pool.tile([128, cs, NO], BF16, tag=f"e2s{cs}c{c0}")
        nc.scalar.activation(e2[:], t2[:], AF.Exp, scale=1.0 / 32.0)
        e2_t[(blk, c0)] = e2

    def z2r2(blk, c0, cs):
        e2 = e2_t[(blk, c0)]
        z2 = spool.tile([128, cs], F32, tag=f"z2s{cs}c{c0}")
        nc.vector.tensor_reduce(z2[:], e2[:], axis=mybir.AxisListType.X, op=OP.add)
        r2 = r2pool.tile([128, cs], F32, tag=f"r2s{cs}c{c0}")
        nc.vector.reciprocal(r2[:], z2[:])
        r2_t[(blk, c0)] = r2

    # weighting route per unit-slot: balance ACT/Pool/DVE under the PE floor
    #  A: ACT copy*r2 -> DVE TT*e2 (2x mode);  P: gpsimd STT;  B: DVE STT
    # route per unit-slot (gpsimd cannot read PSUM on real HW):
    #  B: DVE STT from PSUM;  A: ACT copy*r2 -> DVE TT*e2 (2x);
    #  C: ACT copy*r2 -> Pool STT*e2
    # B on even slots; A/C counts alternate by block parity so ACT/DVE/Pool
    # all amortize just under the PE floor.
    ROUTES_EVEN = ['B', 'A', 'C', 'B', 'A', 'C', 'B', 'A',
                    'C', 'B', 'A', 'C', 'A', 'B', 'C', 'B']
    ROUTES_ODD = ROUTES_EVEN
    # last block: finish with fast DVE evacs so the final E-train isn't gated
    # by a slow Pool multiply
    ROUTES_LAST = ['B', 'A', 'C', 'B', 'A', 'C', 'B', 'A',
                   'C', 'B', 'A', 'C', 'C', 'A', 'B', 'B']

    def emit_back(blk):
        """Pass-2 (ih matmuls, weighting, E-contraction) for one block,
        with the next block's routing stages interleaved at the right spots."""
        nxt = blk + 1 if blk + 1 < NBLOCKS else None
        if blk != 0:
            for c0, cs in chunks_of(blk):
                z2r2(blk, c0, cs)
            # pass1/e1 of the next block lead the PE/ACT queues this cycle,
            # so ps1 (single-buffered) is freed early and e1 is ready for z1.
            if nxt is not None:
                pass1(nxt)
                for c0, cs in chunks_of(nxt):
                    e1_(nxt, c0, cs)
        w_tile = w_tiles[blk]
        q0 = blk * WCHUNK_Q
        # interleave points for next-block routing stages (on their engines)
        if blk == 0:
            hooks = {1: "z2r2@4@4", 5: "z2r2@8@8",
                     6: "p1e1_nxt", 8: "z1r1", 12: "mid", 14: "e2"}
        else:
            hooks = {1: "z1r1", 3: "mid", 6: "e2"}
        if blk == NBLOCKS - 1:
            # drain batch-half h=1 first so the epilogue's h=1 copy/DMA
            # overlaps the remaining h=0 E-contractions
            j_order = [1, 3, 5, 7, 9, 11, 13, 15, 0, 2, 4, 6, 8, 10, 12, 14]
        else:
            j_order = list(range(UNITS_PER_BLOCK))
        for j in j_order:
            u = blk * UNITS_PER_BLOCK + j
            q, h = u // 2, u % 2
            ps_ih = psihpool.tile([128, OD], F32)
            nc.tensor.matmul(
                ps_ih[:],
                lhsT=x_sb[:, q, h, :],
                rhs=w_tile[:, q - q0, :],
                start=True, stop=True,
            )
            # psum free dim is (d, o); e2 broadcast over d has innermost step 1
            k, jj = ckey(blk, j)
            e2_b = e2_t[k][:, jj, None, :].to_broadcast((128, DO, NO))
            r2_s = r2_t[k][:, jj:jj + 1]
            tmp = tmppool.tile([128, DO, NO], BF16, tag="tmp")
            ps_v = ps_ih.rearrange("p (d o) -> p d o", o=NO)
            if blk == NBLOCKS - 1:
                rt = ROUTES_LAST[j]
            else:
                rt = (ROUTES_ODD if blk % 2 else ROUTES_EVEN)[j]
            if rt == 'B':
                nc.vector.scalar_tensor_tensor(
                    tmp[:], ps_v, r2_s, e2_b, op0=OP.mult, op1=OP.mult,
                )
            else:
                ihr = tmppool.tile([128, DO, NO], BF16, tag="ihr")
                nc.scalar.activation(ihr[:], ps_v, AF.Copy, scale=r2_s)
                if rt == 'C':
                    nc.gpsimd.tensor_tensor(tmp[:], ihr[:], e2_b, op=OP.mult)
                else:
                    nc.vector.tensor_tensor(tmp[:], ihr[:], e2_b, op=OP.mult)
            pending_e.append((u, tmp))
            flush_e(False)
            if WARM and blk == 0 and j <= 6:
                warm_mm(128)
                warm_mm(128)
                warm_mm(128)
            stage = hooks.get(j)
            if stage:
                if stage.startswith("z2r2@"):
                    _, zc0, zcs = stage.split("@")
                    z2r2(blk, int(zc0), int(zcs))
                elif nxt is None:
                    pass
                elif stage == "p1e1_nxt":
                    pass1(nxt)
                    for c0, cs in chunks_of(nxt):
                        e1_(nxt, c0, cs)
                else:
                    fn = {"z1r1": z1r1, "mid": mid, "e2": e2_}[stage]
                    for c0, cs in chunks_of(nxt):
                        fn(nxt, c0, cs)

    # prologue: block 0 dmas + chunked routing chain
    nc.gpsimd.memset(warm_sb[:], 0.0)
    if WARM:
        warm_mm(512)
        warm_mm(512)
        warm_mm(512)
    emit_dma_xws(0, split=True)
    emit_dma_w(0)
    emit_dma_xws(1)
    nc.sync.dma_start(es_sb[:], es_d[:])
    emit_dma_w(1)
    pass1(0)
    for c0, cs in chunks_of(0):
        e1_(0, c0, cs)
    for c0, cs in chunks_of(0):
        z1r1(0, c0, cs)
    for c0, cs in chunks_of(0):
        mid(0, c0, cs)
        e2_(0, c0, cs)
    z2r2(0, 0, 4)   # remaining blk-0 chunks are interleaved into back(0)
    emit_dma(2)
    for blk in range(NBLOCKS):
        if blk + 3 < NBLOCKS:
            emit_dma(blk + 3)
        emit_back(blk)
    flush_e(True)

    # ---------------- epilogue: s accumulated directly in ps_s [64, 512] ----
    # two partition-half copies/DMAs; the h=1 half only depends on h=1 Es
    # (subtile deps), which drain first in the reordered last block
    s_sb1 = const.tile([32, OD], F32)
    s_sb0 = const.tile([32, OD], F32)
    nc.scalar.copy(s_sb1[:, :], ps_s[32:64, :])
    nc.vector.tensor_copy(s_sb0[:, :], ps_s[0:32, :])
    nc.sync.dma_start(s_d[32:64, :], s_sb1[:, :])
    nc.sync.dma_start(s_d[0:32, :], s_sb0[:, :])
    ctx.close()


_NC_CACHE = None


def _get_program():
    global _NC_CACHE
    if _NC_CACHE is None:
        _NC_CACHE = _build_program()
    return _NC_CACHE


def kernel(inputs: np.ndarray, W: np.ndarray) -> np.ndarray:
    inputs = np.asarray(inputs, dtype=np.float32)
    W = np.asarray(W, dtype=np.float32)

    bf16 = ml_dtypes.bfloat16
    NQT = NI // 4  # quads over the full Ni
    # x block-diagonal stationaries: [NQT, 2, 4, 16, 4, 32] with blocks on the
    # (g, g) diagonal; block (q, h, g) = inputs[32h:32h+32, 4q+g, :].T
    xt = inputs.transpose(1, 2, 0)            # [Ni, Di, B]
    src = xt.reshape(NQT, 4, DI, 2, 32)       # [q, g, i, h, b]
    x4 = np.zeros((NQT, 2, 4, DI, 4, 32), dtype=np.float32)
    for g in range(4):
        x4[:, :, g, :, g, :] = src[:, g].transpose(0, 2, 1, 3)  # [q, h, i, b]
    x4 = x4.reshape(NQT, 2, 64, 128).transpose(2, 0, 1, 3)      # [64, q, h, 128]
    x4 = np.ascontiguousarray(x4).astype(bf16)
    # W: [1, Ni, No, Do, Di] -> [q, (g,i)=64, Do*No]  (columns are (d,o)-major)
    w4 = np.ascontiguousarray(
        W[0].transpose(0, 3, 2, 1).reshape(NQT, 4 * DI, OD)).astype(bf16)
    # Wsum over Do: [Ni, No, Di] -> [(g,i)=64, q, No]
    ws4 = W[0].sum(axis=2).transpose(0, 2, 1).reshape(NQT, 4 * DI, NO)
    ws4 = np.ascontiguousarray(ws4.transpose(1, 0, 2)).astype(bf16)  # [64, q, No]
    esel = np.tile(np.eye(32, dtype=np.float32), (4, 1)).astype(bf16)

    nc = _get_program()
    in_maps = []
    for c in range(NCORES):
        sl = slice(c * NQ, (c + 1) * NQ)
        in_maps.append({
            "x": np.ascontiguousarray(x4[:, sl]),
            "w": np.ascontiguousarray(w4[sl]),
            "ws": np.ascontiguousarray(ws4[:, sl]),
            "esel": esel,
        })
    res = run_bass_kernel_spmd(nc, in_maps, core_ids=list(range(NCORES)))
    s = np.zeros((64, OD), dtype=np.float32)
    for r in res.results:
        s += np.asarray(r["s_out"], dtype=np.float32)
    s = s.reshape(B, DO, NO).transpose(0, 2, 1)  # -> [B, No, Do]
    s2 = np.sum(np.square(s), axis=-1, keepdims=True)
    scale = s2 / (1.0 + s2) / np.sqrt(s2 + EPS)
    return (scale * s).astype(np.float32)



# revision 90
# speedup vs baseline: 1.0006x; 1.0006x over previous
"""CapsuleLayer (dynamic routing, 3 iterations) Trainium2 Bass kernel.

Problem: inputs [64, 2048, 16] f32, W [1, 2048, 32, 16, 16] f32
  inputs_hat[b,n,o,d] = sum_i W[n,o,d,i] * inputs[b,n,i]
  3 routing iterations (softmax over o); only the last s/squash matters, and the
  b-update never uses `outputs`, so the whole computation collapses to:
    ihsum[b,n,o] = sum_d ih[b,n,o,d]            (= x . Wsum)
    e1 = exp(ihsum/32); c1 = e1 / sum_o e1
    b2 = ihsum*(1/32 + c1); e2 = exp(b2); r2 = 1/sum_o e2   (c2 = e2*r2)
    s[b,o,d] = sum_n c2[b,n,o] * ih[b,n,o,d]
    out = squash(s)

Sharding: Ni (2048) split 8 ways (256 capsules per core). Routing is local per
(b, n); each core produces a partial s [64, 32, 16] which the host sums and
squashes (tiny: 32K elements).

Per-core device pipeline (all matmuls bf16, PSUM f32), software-pipelined
one block (16 units = 8 quads) ahead:
  pass 1: ihsum via per-capsule matmuls, x stationary (4 capsules col-tiled,
          batch split in halves of 32), batched 16 units per PSUM bank.
  routing (staged across the previous block's pass 2; real-HW constraint:
          gpsimd cannot read PSUM and has no fused STT):
          e1 = ACT exp(ps1/32); z1,r1,r1s=32/z1 on DVE;
          v32 = 32*c1 = e1*r1s (gpsimd TT); t2raw = (v32+1)*ihsum
          (DVE fused STT, PSUM src); e2 = ACT exp(t2raw/32); z2,r2 on DVE.
  pass 2: per unit: ih into PSUM [128, 512]; weighting routes balanced
          under the PE floor: B = DVE STT (psum*r2)*e2, A = ACT copy*r2 +
          DVE TT*e2 (2x mode), C = ACT copy*r2 + gpsimd TT*e2; then a PE
          contraction with a fixed 0/1 selector [128->32] accumulating the
          batch-half h's partial s into ps_s[32h:32h+32] (PSUM, 2 groups).
  epilogue: ACT copies ps_s halves to SBUF, DMA to HBM; host sums the 8
          per-core partials and applies squash.
"""

import os
import sys

import numpy as np
import ml_dtypes

sys.path.insert(0, "/opt/trn_rl_repo")
sys.path.insert(0, "/opt/pypackages")

import concourse.bass as bass
import concourse.mybir as mybir
import concourse.tile as tile
from concourse import bacc
from concourse.bass_utils import run_bass_kernel_spmd

BF16 = mybir.dt.bfloat16
F32 = mybir.dt.float32
AF = mybir.ActivationFunctionType
OP = mybir.AluOpType

B, NI, DI, NO, DO = 64, 2048, 16, 32, 16
NCORES = 8
NL = NI // NCORES            # 256 capsules per core
OD = NO * DO                 # 512
NQ = NL // 4                 # 64 quads (4 capsules each)
NUNITS = NQ * 2              # 128 units: (quad, batch-half)
UNITS_PER_BLOCK = 16         # routing block: 16 units -> psum [128, 512]
NBLOCKS = NUNITS // UNITS_PER_BLOCK   # 8
WCHUNK_Q = 8                 # quads per W dma chunk (32 capsules)
EPS = 1e-7
WARM = int(os.environ.get('K_WARM', '1'))
# per 16 units: route 1 = DVE scalar_tensor_tensor (fused), route 2 =
# ACT copy + GPSIMD multiply, route 3 = ACT copy + DVE multiply (2x mode)
ROUTE1 = frozenset(range(8))
ROUTE2 = frozenset({14})


def _build_program():
    nc = bacc.Bacc("TRN2", target_bir_lowering=False, debug=False)

    x_d = nc.dram_tensor("x", [64, NQ, 2, 128], BF16, kind="ExternalInput").ap()
    w_d = nc.dram_tensor("w", [NQ, 64, OD], BF16, kind="ExternalInput").ap()
    ws_d = nc.dram_tensor("ws", [64, NQ, NO], BF16, kind="ExternalInput").ap()
    es_d = nc.dram_tensor("esel", [128, 32], BF16, kind="ExternalInput").ap()
    s_d = nc.dram_tensor("s_out", [64, OD], F32, kind="ExternalOutput").ap()

    with tile.TileContext(nc) as tc:
        _emit(tc, x_d, w_d, ws_d, es_d, s_d)
    nc.compile()
    return nc


def _emit(tc, x_d, w_d, ws_d, es_d, s_d):
    nc = tc.nc
    from contextlib import ExitStack

    ctx = ExitStack()
    const = ctx.enter_context(tc.tile_pool(name="const", bufs=1))
    wpool = ctx.enter_context(tc.tile_pool(name="w", bufs=4))
    rpool = ctx.enter_context(tc.tile_pool(name="routing", bufs=4))
    spool = ctx.enter_context(tc.tile_pool(name="small", bufs=4))
    e2pool = ctx.enter_context(tc.tile_pool(name="e2", bufs=4))
    r2pool = ctx.enter_context(tc.tile_pool(name="r2", bufs=4))
    tmppool = ctx.enter_context(tc.tile_pool(name="tmp", bufs=12))
    ps1pool = ctx.enter_context(tc.tile_pool(name="ps1", bufs=1, space="PSUM"))
    psihpool = ctx.enter_context(tc.tile_pool(name="psih", bufs=6, space="PSUM"))
    psspool = ctx.enter_context(tc.tile_pool(name="pss", bufs=1, space="PSUM"))

    # resident inputs
    # x: block-diagonal stationary per (quad, half): [64 = (4n,16i), 128 = (4n,32b)]
    # loaded in per-block slices, interleaved with W so block k's inputs all
    # arrive ~2 block-periods before use
    x_sb = const.tile([64, NQ, 2, 128], BF16)
    ws_sb = const.tile([64, NQ, NO], BF16)
    es_sb = const.tile([128, 32], BF16)
    warm_sb = const.tile([64, 512], BF16)
    w_tiles = [None] * NBLOCKS

    def emit_dma_xws(blk, split=False):
        q0 = blk * WCHUNK_Q
        if split:
            # halve the first transfer so block 0's first pass-1 matmuls
            # (quads q0..q0+3) start as early as possible
            hq = WCHUNK_Q // 2
            nc.sync.dma_start(x_sb[:, q0:q0 + hq], x_d[:, q0:q0 + hq])
            nc.sync.dma_start(ws_sb[:, q0:q0 + hq], ws_d[:, q0:q0 + hq])
            nc.sync.dma_start(x_sb[:, q0 + hq:q0 + WCHUNK_Q],
                              x_d[:, q0 + hq:q0 + WCHUNK_Q])
            nc.sync.dma_start(ws_sb[:, q0 + hq:q0 + WCHUNK_Q],
                              ws_d[:, q0 + hq:q0 + WCHUNK_Q])
            return
        nc.sync.dma_start(ws_sb[:, q0:q0 + WCHUNK_Q], ws_d[:, q0:q0 + WCHUNK_Q])
        nc.sync.dma_start(x_sb[:, q0:q0 + WCHUNK_Q], x_d[:, q0:q0 + WCHUNK_Q])

    def emit_dma_w(blk, split=False):
        q0 = blk * WCHUNK_Q
        w_tile = wpool.tile([64, WCHUNK_Q, OD], BF16, tag="wt")
        if split:
            hq = WCHUNK_Q // 2
            nc.sync.dma_start(
                w_tile[:, 0:hq],
                w_d[q0:q0 + hq].rearrange("q p f -> p q f"))
            nc.sync.dma_start(
                w_tile[:, hq:WCHUNK_Q],
                w_d[q0 + hq:q0 + WCHUNK_Q].rearrange("q p f -> p q f"))
        else:
            nc.sync.dma_start(
                w_tile[:], w_d[q0:q0 + WCHUNK_Q].rearrange("q p f -> p q f")
            )
        w_tiles[blk] = w_tile

    def emit_dma(blk):
        emit_dma_xws(blk)
        emit_dma_w(blk)

    # s accumulator psum, lives across the whole pass 2.
    # [64 b, 512 = (d,o)]; batch-half h units accumulate partitions 32h:32h+32
    # (disjoint partition ranges -> subtile deps let the epilogue drain halves)
    ps_s = psspool.tile([64, OD], F32)
    s_written = [False, False]
    e_emitted = [0, 0]

    # PE warm-up/filler matmuls: keep the tensor engine busy (and its p-state
    # ramp hot) during pipeline fill. Only legal before the first real E.
    def warm_mm(cols):
        nc.tensor.matmul(
            ps_s[:, 0:cols],
            lhsT=warm_sb[:, 0:64],
            rhs=warm_sb[:, 0:cols],
            start=True, stop=True,
        )

    e2_blocks = [None] * NBLOCKS
    r2_blocks = [None] * NBLOCKS

    # deferred E-contractions: list of (u, tmp_tile)
    E_LAG = 11
    pending_e = []

    def flush_e(u_final, lag=None):
        lag = E_LAG if lag is None else lag
        while pending_e and (len(pending_e) > lag or u_final):
            eu, etmp = pending_e.pop(0)
            h = eu % 2
            e_emitted[h] += 1
            nc.tensor.matmul(
                ps_s[32 * h:32 * (h + 1), :],
                lhsT=es_sb[:],
                rhs=etmp.rearrange("p d o -> p (d o)"),
                start=not s_written[h], stop=(e_emitted[h] == NUNITS // 2),
            )
            s_written[h] = True

    # -------- software pipeline over blocks --------
    # Routing is staged in (blk, chunk) pieces so each in-order engine queue
    # matches data readiness. Block 0 uses 4-unit chunks to cut pipeline-fill
    # latency; steady blocks use one 16-unit chunk per stage:
    #   pass1(b):      ihsum matmuls (PE)
    #   e1_(b,c):      exp (ACT)
    #   z1r1(b,c):     reduce+recip (DVE)
    #   mid(b,c):      u1, t2 (Pool STT)
    #   e2_(b,c):      exp (ACT)
    #   z2r2(b,c):     reduce+recip (DVE)
    ps1_blocks = [None] * NBLOCKS
    e1_t, r1_t, e2_t, r2_t = {}, {}, {}, {}

    def chunks_of(blk):
        if blk == 0:
            return [(0, 4), (4, 4), (8, 8)]
        if blk == 1:
            return [(0, 8), (8, 8)]
        return [(0, 16)]

    def ckey(blk, j):
        if blk == 0:
            c0 = (j // 4) * 4 if j < 8 else 8
        elif blk == 1:
            c0 = (j // 8) * 8
        else:
            c0 = 0
        return (blk, c0), j - c0

    def pass1(blk):
        ps1 = ps1pool.tile([128, UNITS_PER_BLOCK * NO], F32)
        for j in range(UNITS_PER_BLOCK):
            u = blk * UNITS_PER_BLOCK + j
            q, h = u // 2, u % 2
            nc.tensor.matmul(
                ps1[:, 32 * j:32 * (j + 1)],
                lhsT=x_sb[:, q, h, :],
                rhs=ws_sb[:, q, :],
                start=True, stop=True,
            )
        ps1_blocks[blk] = ps1

    def ps1v(blk, c0, cs):
        return ps1_blocks[blk].rearrange(
            "p (j o) -> p j o", o=NO)[:, c0:c0 + cs, :]

    def e1_(blk, c0, cs):
        e1 = rpool.tile([128, cs, NO], BF16, tag=f"e1s{cs}c{c0}")
        nc.scalar.activation(e1[:], ps1v(blk, c0, cs), AF.Exp, scale=1.0 / 32.0)
        e1_t[(blk, c0)] = e1

    def z1r1(blk, c0, cs):
        e1 = e1_t[(blk, c0)]
        z1 = spool.tile([128, cs], F32, tag=f"z1s{cs}c{c0}")
        nc.vector.tensor_reduce(z1[:], e1[:], axis=mybir.AxisListType.X, op=OP.add)
        r1 = spool.tile([128, cs], F32, tag=f"r1s{cs}c{c0}")
        nc.vector.reciprocal(r1[:], z1[:])
        r1s = spool.tile([128, cs], F32, tag=f"r1x{cs}c{c0}")
        nc.vector.tensor_scalar_mul(r1s[:], r1[:], 32.0)
        r1_t[(blk, c0)] = r1s

    def mid(blk, c0, cs):
        # v32 = 32*c1 = e1 * (32*r1)  (gpsimd TT: the only legal Pool form)
        # t2raw = (v32 + 1) * ihsum   (DVE fused STT, reads ihsum from PSUM)
        # e2 = exp(t2raw / 32) = exp(ihsum*(1/32 + c1))
        e1 = e1_t[(blk, c0)]
        r1_b = r1_t[(blk, c0)][:, :, None].to_broadcast((128, cs, NO))
        v32 = rpool.tile([128, cs, NO], BF16, tag=f"u1s{cs}c{c0}")
        nc.gpsimd.tensor_tensor(v32[:], e1[:], r1_b, op=OP.mult)
        t2 = rpool.tile([128, cs, NO], BF16, tag=f"t2s{cs}c{c0}")
        nc.vector.scalar_tensor_tensor(t2[:], v32[:], 1.0, ps1v(blk, c0, cs),
                                       op0=OP.add, op1=OP.mult)
        e1_t[(blk, c0, "t2")] = t2

    def e2_(blk, c0, cs):
        t2 = e1_t[(blk, c0, "t2")]
        e2 = e2pool.tile([128, cs, NO], BF16, tag=f"e2s{cs}c{c0}")
        nc.scalar.activation(e2[:], t2[:], AF.Exp, scale=1.0 / 32.0)
        e2_t[(blk, c0)] = e2

    def z2r2(blk, c0, cs):
        e2 = e2_t[(blk, c0)]
        z2 = spool.tile([128, cs], F32, tag=f"z2s{cs}c{c0}")
        nc.vector.tensor_reduce(z2[:], e2[:], axis=mybir.AxisListType.X, op=OP.add)
        r2 = r2pool.tile([128, cs], F32, tag=f"r2s{cs}c{c0}")
        nc.vector.reciprocal(r2[:], z2[:])
        r2_t[(blk, c0)] = r2

    # weighting route per unit-slot: balance ACT/Pool/DVE under the PE floor
    #  A: ACT copy*r2 -> DVE TT*e2 (2x mode);  P: gpsimd STT;  B: DVE STT
    # route per unit-slot (gpsimd cannot read PSUM on real HW):
    #  B: DVE STT from PSUM;  A: ACT copy*r2 -> DVE TT*e2 (2x);
    #  C: ACT copy*r2 -> Pool STT*e2
    # B on even slots; A/C counts alternate by block parity so ACT/DVE/Pool
    # all amortize just under the PE floor.
    ROUTES_EVEN = ['B', 'A', 'C', 'B', 'A', 'C', 'B', 'A',
                    'C', 'B', 'A', 'C', 'A', 'B', 'C', 'B']
    ROUTES_ODD = ROUTES_EVEN
    # last block: finish with fast DVE evacs so the final E-train isn't gated
    # by a slow Pool multiply
    ROUTES_LAST = ['B', 'A', 'C', 'B', 'A', 'C', 'B', 'A',
                   'C', 'B', 'A', 'C', 'C', 'A', 'B', 'B']

    def emit_back(blk):
        """Pass-2 (ih matmuls, weighting, E-contraction) for one block,
        with the next block's routing stages interleaved at the right spots."""
        nxt = blk + 1 if blk + 1 < NBLOCKS else None
        if blk != 0:
            for c0, cs in chunks_of(blk):
                z2r2(blk, c0, cs)
            # pass1/e1 of the next block lead the PE/ACT queues this cycle,
            # so ps1 (single-buffered) is freed early and e1 is ready for z1.
            if nxt is not None:
                pass1(nxt)
                for c0, cs in chunks_of(nxt):
                    e1_(nxt, c0, cs)
        w_tile = w_tiles[blk]
        q0 = blk * WCHUNK_Q
        # interleave points for next-block routing stages (on their engines)
        if blk == 0:
            hooks = {1: "z2r2@4@4", 5: "z2r2@8@8",
                     6: "p1e1_nxt", 8: "z1r1", 12: "mid", 14: "e2"}
        else:
            hooks = {1: "z1r1", 3: "mid", 6: "e2"}
        if blk == NBLOCKS - 1:
            # drain batch-half h=1 first so the epilogue's h=1 copy/DMA
            # overlaps the remaining h=0 E-contractions
            j_order = [1, 3, 5, 7, 9, 11, 13, 15, 0, 2, 4, 6, 8, 10, 12, 14]
        else:
            j_order = list(range(UNITS_PER_BLOCK))
        for j in j_order:
            u = blk * UNITS_PER_BLOCK + j
            q, h = u // 2, u % 2
            ps_ih = psihpool.tile([128, OD], F32)
            nc.tensor.matmul(
                ps_ih[:],
                lhsT=x_sb[:, q, h, :],
                rhs=w_tile[:, q - q0, :],
                start=True, stop=True,
            )
            # psum free dim is (d, o); e2 broadcast over d has innermost step 1
            k, jj = ckey(blk, j)
            e2_b = e2_t[k][:, jj, None, :].to_broadcast((128, DO, NO))
            r2_s = r2_t[k][:, jj:jj + 1]
            tmp = tmppool.tile([128, DO, NO], BF16, tag="tmp")
            ps_v = ps_ih.rearrange("p (d o) -> p d o", o=NO)
            if blk == NBLOCKS - 1:
                rt = ROUTES_LAST[j]
            else:
                rt = (ROUTES_ODD if blk % 2 else ROUTES_EVEN)[j]
            if rt == 'B':
                nc.vector.scalar_tensor_tensor(
                    tmp[:], ps_v, r2_s, e2_b, op0=OP.mult, op1=OP.mult,
                )
            else:
                ihr = tmppool.tile([128, DO, NO], BF16, tag="ihr")
                nc.scalar.activation(ihr[:], ps_v, AF.Copy, scale=r2_s)
                if rt == 'C':
                    nc.gpsimd.tensor_tensor(tmp[:], ihr[:], e2_b, op=OP.mult)
                else:
                    nc.vector.tensor_tensor(tmp[:], ihr[:], e2_b, op=OP.mult)
            pending_e.append((u, tmp))
            flush_e(False)
            if WARM and blk == 0 and j <= 6:
                warm_mm(128)
                warm_mm(128)
                warm_mm(128)
            stage = hooks.get(j)
            if stage:
                if stage.startswith("z2r2@"):
                    _, zc0, zcs = stage.split("@")
                    z2r2(blk, int(zc0), int(zcs))
                elif nxt is None:
                    pass
                elif stage == "p1e1_nxt":
                    pass1(nxt)
                    for c0, cs in chunks_of(nxt):
                        e1_(nxt, c0, cs)
                else:
                    fn = {"z1r1": z1r1, "mid": mid, "e2": e2_}[stage]
                    for c0, cs in chunks_of(nxt):
                        fn(nxt, c0, cs)

    # prologue: block 0 dmas + chunked routing chain
    nc.gpsimd.memset(warm_sb[:], 0.0)
    if WARM:
        warm_mm(512)
        warm_mm(512)
        warm_mm(512)
    emit_dma_xws(0, split=True)
    emit_dma_w(0, split=True)
    emit_dma_xws(1)
    emit_dma_w(1)
    nc.sync.dma_start(es_sb[:], es_d[:])
    pass1(0)
    for c0, cs in chunks_of(0):
        e1_(0, c0, cs)
    for c0, cs in chunks_of(0):
        z1r1(0, c0, cs)
    for c0, cs in chunks_of(0):
        mid(0, c0, cs)
        e2_(0, c0, cs)
    z2r2(0, 0, 4)   # remaining blk-0 chunks are interleaved into back(0)
    emit_dma(2)
    for blk in range(NBLOCKS):
        if blk + 3 < NBLOCKS:
            emit_dma(blk + 3)
        emit_back(blk)
    flush_e(True)

    # ---------------- epilogue: s accumulated directly in ps_s [64, 512] ----
    # two partition-half copies/DMAs; the h=1 half only depends on h=1 Es
    # (subtile deps), which drain first in the reordered last block
    s_sb1 = const.tile([32, OD], F32)
    s_sb0 = const.tile([32, OD], F32)
    nc.scalar.copy(s_sb1[:, :], ps_s[32:64, :])
    nc.vector.tensor_copy(s_sb0[:, :], ps_s[0:32, :])
    nc.sync.dma_start(s_d[32:64, :], s_sb1[:, :])
    nc.sync.dma_start(s_d[0:32, :], s_sb0[:, :])
    ctx.close()


_NC_CACHE = None


def _get_program():
    global _NC_CACHE
    if _NC_CACHE is None:
        _NC_CACHE = _build_program()
    return _NC_CACHE


def kernel(inputs: np.ndarray, W: np.ndarray) -> np.ndarray:
    inputs = np.asarray(inputs, dtype=np.float32)
    W = np.asarray(W, dtype=np.float32)

    bf16 = ml_dtypes.bfloat16
    NQT = NI // 4  # quads over the full Ni
    # x block-diagonal stationaries: [NQT, 2, 4, 16, 4, 32] with blocks on the
    # (g, g) diagonal; block (q, h, g) = inputs[32h:32h+32, 4q+g, :].T
    xt = inputs.transpose(1, 2, 0)            # [Ni, Di, B]
    src = xt.reshape(NQT, 4, DI, 2, 32)       # [q, g, i, h, b]
    x4 = np.zeros((NQT, 2, 4, DI, 4, 32), dtype=np.float32)
    for g in range(4):
        x4[:, :, g, :, g, :] = src[:, g].transpose(0, 2, 1, 3)  # [q, h, i, b]
    x4 = x4.reshape(NQT, 2, 64, 128).transpose(2, 0, 1, 3)      # [64, q, h, 128]
    x4 = np.ascontiguousarray(x4).astype(bf16)
    # W: [1, Ni, No, Do, Di] -> [q, (g,i)=64, Do*No]  (columns are (d,o)-major)
    w4 = np.ascontiguousarray(
        W[0].transpose(0, 3, 2, 1).reshape(NQT, 4 * DI, OD)).astype(bf16)
    # Wsum over Do: [Ni, No, Di] -> [(g,i)=64, q, No]
    ws4 = W[0].sum(axis=2).transpose(0, 2, 1).reshape(NQT, 4 * DI, NO)
    ws4 = np.ascontiguousarray(ws4.transpose(1, 0, 2)).astype(bf16)  # [64, q, No]
    esel = np.tile(np.eye(32, dtype=np.float32), (4, 1)).astype(bf16)

    nc = _get_program()
    in_maps = []
    for c in range(NCORES):
        sl = slice(c * NQ, (c + 1) * NQ)
        in_maps.append({
            "x": np.ascontiguousarray(x4[:, sl]),
            "w": np.ascontiguousarray(w4[sl]),
            "ws": np.ascontiguousarray(ws4[:, sl]),
            "esel": esel,
        })
    res = run_bass_kernel_spmd(nc, in_maps, core_ids=list(range(NCORES)))
    s = np.zeros((64, OD), dtype=np.float32)
    for r in res.results:
        s += np.asarray(r["s_out"], dtype=np.float32)
    s = s.reshape(B, DO, NO).transpose(0, 2, 1)  # -> [B, No, Do]
    s2 = np.sum(np.square(s), axis=-1, keepdims=True)
    scale = s2 / (1.0 + s2) / np.sqrt(s2 + EPS)
    return (scale * s).astype(np.float32)

